# revision 34
# baseline (speedup 1.0000x reference)
"""Trainium2 Bass kernel for a GNN message-passing layer (BoundaryConvLayer).

Computation (reference, per node i over D=128 channels):
    rate  = softplus(x @ W_rate) + EPS
    gamma = x @ W_rob + b_rob
    h     = x @ W_fc + b_fc
    agg   = segment_sum(h[row] + h[col], row)
    y     = LayerNorm((rate*agg + gamma) / (1 + rate*deg + EPS)) * ln_gamma + ln_beta

Distribution: nodes sharded across 8 cores by contiguous row blocks; edges
partitioned by destination row so the segment sum is local.

Design (v2):
  * Per-core COMPACT gather table: only the ~63% of nodes actually referenced
    as sources by this core's edges are materialized (h = x@W_fc + b_fc rows,
    bf16), split into NCHK=2 chunks so dma_gather's int16 indices reach all
    rows.  Phase 1 computes the table with pipelined 512-col PSUM groups and
    writes chunk-q rows to a chunk-q-only DRAM tensor, so chunk-0 gathers
    start halfway through phase 1.
  * agg identity: agg[i] = cnt[i]*h[i] + sum_{e:row=i} h[col_e] where
    cnt = in-edge count.  The neighbor sum is a one-hot "selection matrix"
    matmul accumulated in PSUM over gathered edge rows; the self term is
    pre-written into the PSUM accumulator by a Scalar-engine copy with a
    per-partition cnt scale (no diag matrix, no extra matmul).
  * Eltwise: reciprocal via the fast custom-DVE approx; per-tile row sums via
    accum_out on STT/ACT; final (y-mean)*rstd applied by a Scalar ACT with
    per-partition scale/bias.  Consts load first on the Scalar queue so
    nothing in phase 3 waits on phase-1 DMA positions.

Gather layout: per tile (128 dst rows) and chunk there are Cq 128-slot
groups; each (chunk, G-tile-group) pair is one dense dma_gather.  Pad slots
point at row 0 and are killed by the zero rows of the selection matrix.
"""

import numpy as np
import ml_dtypes
from contextlib import ExitStack
from dataclasses import dataclass

import concourse.bass as bass
import concourse.tile as tile
from concourse import bacc, mybir
from concourse.bass_utils import run_bass_kernel_spmd

# The stock ACT-table chooser greedily picks the first set containing each
# function, which for {Exp, Ln, Copy, Square, Identity} can alternate between
# sets and reload the table (~1.3us each).  Restrict it to the one set that
# contains all of them so a single load suffices.
_ACT_KEEP = "natural_log_exp_and_others"
if not getattr(bacc, "_act_tables_patched", False):
    _orig_get_tables = bacc.get_activation_tables

    def _patched_get_tables(arch):
        t = _orig_get_tables(arch)
        if _ACT_KEEP in t:
            t = {k: (v if k == _ACT_KEEP else set()) for k, v in t.items()}
        return t

    bacc.get_activation_tables = _patched_get_tables
    bacc._act_tables_patched = True

BF16 = ml_dtypes.bfloat16
EPS = 1e-4
LN_EPS = 1e-5
P = 128
D = 128

# feature flags (bisect aids; all-False falls back to baseline-style ops)
USE_FAST_RECIP = True    # custom-DVE reciprocal_approx_fast
USE_ACCUM = True         # accum_out row sums on STT/ACT
USE_ACT_FINAL = True     # per-partition scale/bias ACT for (y-mean)*rstd


@dataclass
class Cfg:
    N: int            # total nodes
    E: int            # total edges
    NC: int           # cores
    NCHK: int = 2     # gather table chunks (int16 range)
    CPAD: int = 32256 # rows per chunk (252*128, < 32768 for int16 idx)
    Cq: int = 0       # 128-slot groups per (tile, chunk); set by prep
    ln_trivial: bool = False

    @property
    def NLOC(self):
        return self.N // self.NC

    @property
    def T(self):
        return (self.NLOC + P - 1) // P

    @property
    def TLP(self):
        return self.T * P

    @property
    def NCOL(self):  # x-compact table columns
        return self.NCHK * self.CPAD

    @property
    def G(self):      # tiles per gather group
        for g in (7, 14, 4, 2, 1):
            if self.T % g == 0:
                return g
        return 1


def prep(x, edge_index, degree, W_fc, b_fc, W_rate, W_rob, b_rob, ln_gamma, ln_beta,
         cfg: Cfg):
    """Host-side preprocessing: shard + build per-core compact gather tables."""
    N, NC, NCHK, CPAD = cfg.N, cfg.NC, cfg.NCHK, cfg.CPAD
    NLOC, T, TLP = cfg.NLOC, cfg.T, cfg.TLP

    x = np.asarray(x, np.float32)
    edge_index = np.asarray(edge_index, np.int64)
    degree = np.asarray(degree)
    row, col = edge_index[0], edge_index[1]
    xT = x.T.astype(BF16)  # [D, N]

    w_fc = np.ascontiguousarray(W_fc, dtype=np.float32).astype(BF16)
    w_rt = np.ascontiguousarray(W_rate, dtype=np.float32).astype(BF16)
    w_rb = np.ascontiguousarray(W_rob, dtype=np.float32).astype(BF16)
    # table is bias-free (g = x@W_fc); the self-term chain uses g_i + 2*b_fc so
    # cnt*(g_i + 2*b_fc) + sum_slots g[col] == cnt*h_i + sum h[col] exactly
    bfc2 = 2.0 * np.asarray(b_fc, np.float32).reshape(1, D)
    brob = np.asarray(b_rob, np.float32).reshape(1, D)
    onesr = np.ones((1, P), np.float32)

    cfg.ln_trivial = bool(np.all(np.asarray(ln_gamma) == 1.0)
                          and np.all(np.asarray(ln_beta) == 0.0))
    lnab = np.zeros((P, 2 * D), np.float32)
    lnab[:, :D] = np.asarray(ln_gamma, np.float32)[None, :]
    lnab[:, D:] = np.asarray(ln_beta, np.float32)[None, :]

    core_of = row // NLOC

    # pass 1: per-core unique sources + per-(tile,chunk) counts fix global Cq
    percore = []
    maxslots = 0
    for r in range(NC):
        m = core_of == r
        rl = row[m] - r * NLOC
        ce = col[m]
        uniq, cid = np.unique(ce, return_inverse=True)
        assert (len(uniq) + 1) // 2 <= CPAD, (r, len(uniq))
        q_e = (cid & 1).astype(np.int64)
        rowin_e = (cid >> 1).astype(np.int64)
        t_e = rl // P
        cnt_tq = np.bincount(t_e * NCHK + q_e, minlength=T * NCHK).reshape(T, NCHK)
        maxslots = max(maxslots, int(cnt_tq.max()))
        percore.append((rl, uniq, q_e, rowin_e, t_e, cnt_tq))
    Cq = max(1, -(-maxslots // P))
    cfg.Cq = Cq
    G = cfg.G
    NG = T // G
    IPG = G * Cq * P

    in_maps = []
    for r in range(NC):
        rl, uniq, q_e, rowin_e, t_e, cnt_tq = percore[r]

        # x-compact: chunk q of the table holds source u at column
        # q*CPAD + (u>>1); unused tail columns stay zero.
        xTc = np.zeros((P, NCHK * CPAD), BF16)
        uidx = np.arange(len(uniq))
        xTc[:, (uidx & 1) * CPAD + (uidx >> 1)] = xT[:, uniq]

        # order edges by (tile, chunk, SOURCE row) so each gather run reads
        # ascending addresses (HBM row-buffer locality)
        order = np.lexsort((rowin_e, q_e, t_e))
        rl_s, q_s, rw_s, t_s = rl[order], q_e[order], rowin_e[order], t_e[order]
        tq_s = t_s * NCHK + q_s
        run_start = np.zeros(T * NCHK + 1, np.int64)
        np.cumsum(cnt_tq.reshape(-1), out=run_start[1:])
        pos = np.arange(len(rl_s)) - run_start[tq_s]
        tl_s = t_s % G
        gg_s = t_s // G
        ipos = tl_s * (Cq * P) + pos
        idx16 = np.zeros((NCHK, NG, IPG), np.int16)  # pad -> row 0 (sel kills it)
        idx16[q_s, gg_s, ipos] = rw_s.astype(np.int16)
        # wrap each stream: idx i -> [i%16, i//16], replicate to 128 partitions
        idxw = idx16.reshape(NCHK, NG, IPG // 16, 16).transpose(0, 1, 3, 2)
        idxw = np.ascontiguousarray(idxw)
        idxw = np.tile(idxw, (1, 1, 8, 1))           # [NCHK, NG, 128, IPG//16]
        idx_sb = np.ascontiguousarray(
            idxw.transpose(2, 0, 1, 3)).reshape(P, NCHK * NG * (IPG // 16))

        # rowsr: rebased dst row (node % 128) per slot, -1 for pads
        rowsr = np.full((P, T * NCHK * Cq), -1.0, BF16)
        slot_col = t_s * (NCHK * Cq) + q_s * Cq + pos // P
        rowsr[pos % P, slot_col] = (rl_s % P).astype(BF16)

        iotab = np.broadcast_to(
            np.tile(np.arange(P, dtype=BF16)[None, :], (1, NCHK * Cq)),
            (P, NCHK * Cq * P)).copy()

        cnt = np.bincount(rl, minlength=TLP)
        cntb = cnt.astype(np.float32).reshape(T, P).T.copy()
        degl = np.zeros(TLP, np.float32)
        degl[:NLOC] = degree[r * NLOC:(r + 1) * NLOC].astype(np.float32)
        degf = degl.reshape(T, P).T.copy()
        xTloc = np.zeros((P, TLP), BF16)
        xTloc[:, :NLOC] = xT[:, r * NLOC:(r + 1) * NLOC]

        in_maps.append({
            "xTc": xTc, "xTloc": xTloc,
            "Wfc": w_fc, "Wrt": w_rt, "Wrb": w_rb,
            "bfc2": bfc2, "brob": brob, "onesr": onesr, "lnab": lnab,
            "iotab": iotab, "rowsr": rowsr, "idxs": idx_sb,
            "cntb": cntb, "degf": degf, "ident": np.eye(P, dtype=BF16),
        })
    return in_maps


def build(cfg: Cfg):
    """Build the SPMD Bass program (identical on every core)."""
    NC, T, TLP = cfg.NC, cfg.T, cfg.TLP
    NCHK, Cq, CPAD, NCOL = cfg.NCHK, cfg.Cq, cfg.CPAD, cfg.NCOL
    G = cfg.G
    NG = T // G
    IPG = G * Cq * P
    SELW = NCHK * Cq * P       # sel width per tile
    bf = mybir.dt.bfloat16
    f32 = mybir.dt.float32
    i16 = mybir.dt.int16
    B = 4                      # tiles per eltwise batch

    nc = bacc.Bacc("TRN2", target_bir_lowering=False, debug=False, num_devices=NC,
                   num_swdge_queues=4)
    # pre-create ACT bias consts so no memsets land mid-loop
    for val in (LN_EPS, 0.0, 1.0):
        if (f32, val) in nc.const_aps.aps:
            continue
        cs = nc.alloc_sbuf_tensor(f"const-float32-{val}", [P, 1], f32)
        nc.gpsimd.memset(cs.ap(), val)
        nc.const_aps.aps[(f32, val)] = cs.ap()
    nc.all_engine_barrier()

    d_xTc = nc.dram_tensor("xTc", [P, NCOL], bf, kind="ExternalInput").ap()
    d_xTloc = nc.dram_tensor("xTloc", [P, TLP], bf, kind="ExternalInput").ap()
    d_wfc = nc.dram_tensor("Wfc", [P, D], bf, kind="ExternalInput").ap()
    d_wrt = nc.dram_tensor("Wrt", [P, D], bf, kind="ExternalInput").ap()
    d_wrb = nc.dram_tensor("Wrb", [P, D], bf, kind="ExternalInput").ap()
    d_bfc2 = nc.dram_tensor("bfc2", [1, D], f32, kind="ExternalInput").ap()
    d_brob = nc.dram_tensor("brob", [1, D], f32, kind="ExternalInput").ap()
    d_ones = nc.dram_tensor("onesr", [1, P], f32, kind="ExternalInput").ap()
    d_lnab = nc.dram_tensor("lnab", [P, 2 * D], f32, kind="ExternalInput").ap()
    d_iota = nc.dram_tensor("iotab", [P, SELW], bf, kind="ExternalInput").ap()
    d_rowsr = nc.dram_tensor("rowsr", [P, T * NCHK * Cq], bf,
                             kind="ExternalInput").ap()
    d_idxs = nc.dram_tensor("idxs", [P, NCHK * NG * (IPG // 16)], i16,
                            kind="ExternalInput").ap()
    d_cntb = nc.dram_tensor("cntb", [P, T], f32, kind="ExternalInput").ap()
    d_ident = nc.dram_tensor("ident", [P, P], bf, kind="ExternalInput").ap()
    d_degf = nc.dram_tensor("degf", [P, T], f32, kind="ExternalInput").ap()
    # one h-table tensor per chunk so chunk-q gathers depend only on chunk-q
    # phase-1 writes
    d_hq = [nc.dram_tensor(f"htab{q}", [CPAD, D], bf, kind="Internal").ap()
            for q in range(NCHK)]
    d_y = nc.dram_tensor("y", [TLP, D], f32, kind="ExternalOutput").ap()

    with tile.TileContext(nc) as tc, ExitStack() as ctx:
        from concourse import library_config
        nc.gpsimd.load_library(library_config.mlp)

        # ------------- consts: load FIRST, on the Scalar queue -------------
        consts = ctx.enter_context(tc.tile_pool(name="consts", bufs=1))
        wfc = consts.tile([P, D], bf)
        nc.scalar.dma_start(wfc[:], d_wfc[:])
        wrt = consts.tile([P, D], bf)
        nc.scalar.dma_start(wrt[:], d_wrt[:])
        wrb = consts.tile([P, D], bf)
        nc.scalar.dma_start(wrb[:], d_wrb[:])
        bfc2r = consts.tile([1, D], f32)
        nc.scalar.dma_start(bfc2r[:], d_bfc2[:])
        brobr = consts.tile([1, D], f32)
        nc.scalar.dma_start(brobr[:], d_brob[:])
        onesr = consts.tile([1, P], f32)
        nc.scalar.dma_start(onesr[:], d_ones[:])
        iota = consts.tile([P, SELW], bf)
        nc.scalar.dma_start(iota[:], d_iota[:])
        rowsr = consts.tile([P, T * NCHK * Cq], bf)
        nc.scalar.dma_start(rowsr[:], d_rowsr[:])
        idxs = consts.tile([P, NCHK * NG * (IPG // 16)], i16)
        nc.scalar.dma_start(idxs[:], d_idxs[:])
        cntb = consts.tile([P, T], f32)
        nc.scalar.dma_start(cntb[:], d_cntb[:])
        degf = consts.tile([P, T], f32)
        nc.scalar.dma_start(degf[:], d_degf[:])
        xloc = consts.tile([P, TLP], bf)
        nc.scalar.dma_start(xloc[:], d_xTloc[:])
        ident = consts.tile([P, P], bf)
        nc.scalar.dma_start(ident[:], d_ident[:])
        lnab = None
        if not cfg.ln_trivial:
            lnab = consts.tile([P, 2 * D], f32)
            nc.scalar.dma_start(lnab[:], d_lnab[:])

        # ---------------- phase 1: g = x @ W_fc table ----------------
        CHUNK = 8192
        STG = 2048
        GRP = 512
        with tc.tile_pool(name="p1x", bufs=3) as p1x, \
             tc.tile_pool(name="p1ps", bufs=4, space="PSUM") as p1ps, \
             tc.tile_pool(name="p1st", bufs=4) as p1st:
            evac = 0
            for q in range(NCHK):
                for c0 in range(0, CPAD, CHUNK):
                    cw = min(CHUNK, CPAD - c0)
                    xc = p1x.tile([P, CHUNK], bf, tag="xc", name="xc")
                    nc.sync.dma_start(xc[:, :cw],
                                      d_xTc[:, q * CPAD + c0:q * CPAD + c0 + cw])
                    for g0 in range(0, cw, STG):
                        gw = min(STG, cw - g0)
                        gst = p1st.tile([P, STG], bf, tag="gst", name="gst")
                        for s in range(0, gw, GRP):
                            gps = p1ps.tile([P, GRP], f32, space="PSUM",
                                            tag="gps", name="gps")
                            for j in range(0, GRP, P):
                                nc.tensor.matmul(
                                    out=gps[:, j:j + P],
                                    lhsT=xc[:, g0 + s + j:g0 + s + j + P],
                                    rhs=wfc[:],
                                    start=True, stop=True,
                                )
                            if evac & 1:
                                nc.scalar.copy(gst[:, s:s + GRP], gps[:])
                            else:
                                nc.vector.tensor_scalar_mul(
                                    out=gst[:, s:s + GRP], in0=gps[:],
                                    scalar1=1.0)
                            evac += 1
                        dst = d_hq[q][c0 + g0:c0 + g0 + gw, :].rearrange(
                            "(t p) d -> p t d", p=P)
                        nc.scalar.dma_start(dst, gst[:, :gw].rearrange(
                            "p (t d) -> p t d", d=D))

        # ---------------- phase 3: message passing + elementwise -------------
        msgp = ctx.enter_context(tc.tile_pool(name="msgp", bufs=6))
        selp = ctx.enter_context(tc.tile_pool(name="selp", bufs=8))
        eltp = ctx.enter_context(tc.tile_pool(name="eltp", bufs=2))
        smallp = ctx.enter_context(tc.tile_pool(name="smallp", bufs=2))
        apsp = ctx.enter_context(tc.tile_pool(name="apsp", bufs=2, space="PSUM"))
        rpsp = ctx.enter_context(tc.tile_pool(name="rpsp", bufs=2, space="PSUM"))
        gpsp = ctx.enter_context(tc.tile_pool(name="gpsp", bufs=2, space="PSUM"))
        hlp = ctx.enter_context(tc.tile_pool(name="hlp", bufs=2, space="PSUM"))

        def eltwise(bt, tiles):
            nb = len(tiles)
            rps4, aps4, gps4 = bt
            r3 = rps4[:, :nb, :]
            a3 = aps4[:, :nb, :]
            g3 = gps4[:, :nb, :]
            spt = eltp.tile([P, B, D], f32, tag="spt", name="spt")[:, :nb, :]
            rate = eltp.tile([P, B, D], f32, tag="rate", name="rate")[:, :nb, :]
            num = eltp.tile([P, B, D], f32, tag="num", name="num")[:, :nb, :]
            den = eltp.tile([P, B, D], f32, tag="den", name="den")[:, :nb, :]
            y0 = eltp.tile([P, B, D], f32, tag="y0", name="y0")
            sq = eltp.tile([P, B, D], f32, tag="sq", name="sq")
            yf = eltp.tile([P, B, D], f32, tag="yf", name="yf")
            st = smallp.tile([P, 8 * B], f32, tag="st", name="st")
            s1 = st[:, 0:B]
            s2 = st[:, B:2 * B]
            mean = st[:, 2 * B:2 * B + nb]
            msq = st[:, 3 * B:3 * B + nb]
            var = st[:, 4 * B:4 * B + nb]
            rstd = st[:, 5 * B:5 * B + nb]
            nb2 = st[:, 6 * B:6 * B + nb]

            # rate = softplus(x@Wrt) = ln(exp(z)+1); exp/ln share one ACT table
            nc.scalar.activation(out=spt, in_=r3,
                                 func=mybir.ActivationFunctionType.Exp)
            nc.scalar.activation(out=rate, in_=spt,
                                 func=mybir.ActivationFunctionType.Ln,
                                 bias=1.0)
            nc.vector.scalar_tensor_tensor(
                out=num, in0=rate, scalar=EPS, in1=a3,
                op0=mybir.AluOpType.add, op1=mybir.AluOpType.mult)
            nc.vector.tensor_add(out=num, in0=num, in1=g3)
            t0g = tiles[0]
            degb = degf[:, t0g:t0g + nb][:, :, None].to_broadcast([P, nb, D])
            nc.vector.scalar_tensor_tensor(
                out=den, in0=rate, scalar=EPS, in1=degb,
                op0=mybir.AluOpType.add, op1=mybir.AluOpType.mult)
            nc.vector.tensor_scalar_add(out=den, in0=den, scalar1=1.0 + EPS)
            if USE_FAST_RECIP:
                nc.vector.reciprocal_approx_fast(out=den, in_=den)
            else:
                nc.vector.reciprocal(out=den, in_=den)
            y03 = y0[:, :nb, :]
            if USE_ACCUM:
                for jj in range(nb):
                    nc.vector.scalar_tensor_tensor(
                        out=y0[:, jj, :], in0=num[:, jj, :], scalar=1.0,
                        in1=den[:, jj, :],
                        op0=mybir.AluOpType.mult, op1=mybir.AluOpType.mult,
                        accum_out=s1[:, jj:jj + 1])
                    nc.scalar.activation(
                        out=sq[:, jj, :], in_=y0[:, jj, :],
                        func=mybir.ActivationFunctionType.Square,
                        accum_out=s2[:, jj:jj + 1])
            else:
                nc.vector.tensor_mul(out=y03, in0=num, in1=den)
                nc.scalar.square(sq[:, :nb, :], y03)
                nc.vector.tensor_reduce(out=s1[:, :nb], in_=y03,
                                        axis=mybir.AxisListType.X,
                                        op=mybir.AluOpType.add)
                nc.vector.tensor_reduce(out=s2[:, :nb], in_=sq[:, :nb, :],
                                        axis=mybir.AxisListType.X,
                                        op=mybir.AluOpType.add)
            nc.vector.tensor_scalar_mul(out=mean, in0=s1[:, :nb], scalar1=1.0 / D)
            nc.vector.tensor_scalar_mul(out=msq, in0=s2[:, :nb], scalar1=1.0 / D)
            nc.vector.tensor_tensor(out=var, in0=mean, in1=mean,
                                    op=mybir.AluOpType.mult)
            nc.vector.tensor_sub(out=var, in0=msq, in1=var)
            # rstd = (var+eps)^-0.5 = exp(-0.5*ln(var+eps))
            nc.scalar.activation(out=var, in_=var,
                                 func=mybir.ActivationFunctionType.Ln,
                                 bias=LN_EPS)
            nc.scalar.activation(out=rstd, in_=var,
                                 func=mybir.ActivationFunctionType.Exp,
                                 scale=-0.5)
            if USE_ACT_FINAL:
                nc.vector.scalar_tensor_tensor(
                    out=nb2, in0=mean, scalar=-1.0, in1=rstd,
                    op0=mybir.AluOpType.mult, op1=mybir.AluOpType.mult)
                for jj in range(nb):
                    nc.scalar.activation(
                        out=yf[:, jj, :], in_=y0[:, jj, :],
                        func=mybir.ActivationFunctionType.Identity,
                        bias=nb2[:, jj:jj + 1], scale=rstd[:, jj:jj + 1])
            else:
                meanb = mean[:, :, None].to_broadcast([P, nb, D])
                rstdb = rstd[:, :, None].to_broadcast([P, nb, D])
                yf3 = yf[:, :nb, :]
                nc.vector.tensor_sub(out=yf3, in0=y03, in1=meanb)
                nc.vector.tensor_mul(out=yf3, in0=yf3, in1=rstdb)
            if lnab is not None:
                yf3 = yf[:, :nb, :]
                lg = lnab[:, 0:D][:, None, :].to_broadcast([P, nb, D])
                lb = lnab[:, D:2 * D][:, None, :].to_broadcast([P, nb, D])
                nc.vector.tensor_mul(out=yf3, in0=yf3, in1=lg)
                nc.vector.tensor_add(out=yf3, in0=yf3, in1=lb)
            n0 = tiles[0] * P
            nw = nb * P
            dst = d_y[n0:n0 + nw, :].rearrange("(t p) d -> p t d", p=P)
            nc.sync.dma_start(dst, yf[:, :nb, :])

        bt = None
        for gg in range(NG):
            tg0 = gg * G
            # msg layout: [P, q, tile-in-group, c, D] -- q outermost so each
            # chunk's gather writes one contiguous [P, G*Cq, D] section
            msg = msgp.tile([P, NCHK * G * Cq * D], bf, tag="msg", name="msg")
            # split each chunk's gather in two on different SWDGE queues: the
            # gather DMA is latency-bound per queue (~32 GB/s), so saturating
            # all 4 queues quadruples aggregate gather throughput
            GCq = G * Cq
            half = [(0, (GCq + 1) // 2), ((GCq + 1) // 2, GCq // 2)]
            for q in range(NCHK):
                icol = (q * NG + gg) * (IPG // 16)
                base = q * GCq * D
                for si, (s0, ns) in enumerate(half):
                    sec = msg[:, base + s0 * D:base + (s0 + ns) * D]
                    nc.gpsimd.dma_gather(
                        out_ap=sec.rearrange("p (s d) -> p s d", d=D),
                        in_ap=d_hq[q][:],
                        idxs_ap=idxs[:, icol + s0 * 8:icol + (s0 + ns) * 8],
                        num_idxs=ns * P,
                        num_idxs_reg=ns * P,
                        elem_size=D,
                        single_packet=False,
                        queue_num=2 * q + si,
                    )
            for tl in range(G):
                t = tg0 + tl
                j = t % B
                if j == 0:
                    bt = (rpsp.tile([P, B, D], f32, space="PSUM", tag="rps4",
                                    name="rps4"),
                          apsp.tile([P, B, D], f32, space="PSUM", tag="aps4",
                                    name="aps4"),
                          gpsp.tile([P, B, D], f32, space="PSUM", tag="gps4",
                                    name="gps4"))
                rps4, aps4, gps4 = bt
                sel = selp.tile([P, SELW], bf, tag="sel", name="sel")
                rb = rowsr[:, t * NCHK * Cq:(t + 1) * NCHK * Cq][:, :, None] \
                    .to_broadcast([P, NCHK * Cq, P])
                nc.vector.tensor_tensor(
                    out=sel.rearrange("p (c m) -> p c m", c=NCHK * Cq), in0=rb,
                    in1=iota.rearrange("p (c m) -> p c m", c=NCHK * Cq),
                    op=mybir.AluOpType.is_equal)
                # local h rows for the self term: recompute on the PE, scale
                # by cnt[i] on the PSUM evac, accumulate via identity matmul
                hl = hlp.tile([P, D], f32, space="PSUM", tag="hl", name="hl")
                nc.tensor.matmul(out=hl[:], lhsT=xloc[:, t * P:(t + 1) * P],
                                 rhs=wfc[:], start=True, stop=False)
                nc.tensor.matmul(out=hl[:], lhsT=onesr[0:1, :],
                                 rhs=bfc2r[0:1, :], start=False, stop=True)
                hlc = selp.tile([P, D], bf, tag="hlc", name="hlc")
                nc.scalar.activation(out=hlc[:], in_=hl[:],
                                     func=mybir.ActivationFunctionType.Copy,
                                     scale=cntb[:, t:t + 1])
                nc.tensor.matmul(out=aps4[:, j, :], lhsT=ident[:], rhs=hlc[:],
                                 start=True, stop=False)
                for q in range(NCHK):
                    for c in range(Cq):
                        cc = q * Cq + c
                        moff = ((q * G + tl) * Cq + c) * D
                        nc.tensor.matmul(
                            out=aps4[:, j, :], lhsT=sel[:, cc * P:(cc + 1) * P],
                            rhs=msg[:, moff:moff + D],
                            start=False, stop=(cc == NCHK * Cq - 1))
                nc.tensor.matmul(out=rps4[:, j, :],
                                 lhsT=xloc[:, t * P:(t + 1) * P],
                                 rhs=wrt[:], start=True, stop=True)
                nc.tensor.matmul(out=gps4[:, j, :],
                                 lhsT=xloc[:, t * P:(t + 1) * P],
                                 rhs=wrb[:], start=True, stop=False)
                nc.tensor.matmul(out=gps4[:, j, :], lhsT=onesr[0:1, :],
                                 rhs=brobr[0:1, :], start=False, stop=True)
                if j == B - 1 or t == T - 1:
                    eltwise(bt, list(range(t - j, t + 1)))

    nc.compile()
    return nc


def run(inputs, cfg: Cfg, core_ids=None):
    in_maps = prep(**inputs, cfg=cfg)
    nc = build(cfg)
    res = run_bass_kernel_spmd(nc, in_maps, core_ids=core_ids or list(range(cfg.NC)))
    ys = [res.results[r]["y"][:cfg.NLOC] for r in range(cfg.NC)]
    return np.concatenate(ys, axis=0)


def kernel(**inputs):
    cfg = Cfg(N=100_000, E=800_000, NC=8)
    return run(inputs, cfg)
